# revision 1
# baseline (speedup 1.0000x reference)
"""BiLSTM (H=64, input_size=1) + scalar fc head, on 8 Trainium2 NeuronCores.

Sharding: data-parallel over batch (B=1024 -> 128 per core), weights
replicated. Per core the 128-batch is split into NG=2 groups of 64 so the
two independent recurrence chains hide per-op latency.

Layout ("pair-packed"): fwd/bwd LSTM stacked on the partition axis
(rows 0:64 fwd, 64:128 bwd), batch on the free axis. Gate matmuls use
block-diagonal weights so one matmul produces a gate for both directions.
Input/bias terms use K=4 matmuls against host-interleaved [x_t;1;x_rev;1]
tiles. The fc head is one K=128 matmul per step writing one PSUM row per
timestep (both directions reduced in the same matmul).
"""

import os
import sys

import numpy as np

for _p in ("/opt/trn_rl_repo",):
    if os.path.isdir(_p) and _p not in sys.path:
        sys.path.insert(0, _p)

import ml_dtypes  # noqa: E402

import concourse.bass as bass  # noqa: E402
import concourse.bacc as bacc  # noqa: E402
import concourse.tile as tile  # noqa: E402
import concourse.mybir as mybir  # noqa: E402
from concourse.bass_utils import run_bass_kernel_spmd  # noqa: E402

H = 64
NCORES = 8
BLOCAL = 128           # batch rows per core
NG = 2                 # independent batch groups per core
BG = BLOCAL // NG      # 64
OCH = 512              # timesteps per output psum bank (one f32 bank = 512 cols)

DT = mybir.dt.bfloat16
F32 = mybir.dt.float32
AF = mybir.ActivationFunctionType
BF16 = ml_dtypes.bfloat16

# gate col-block order inside the psum tile: sigmoid on I,F,O then tanh on G
GATE_ORDER = ("I", "F", "O", "G")
GATE_OFFSET = {"I": 0, "F": 64, "G": 128, "O": 192}  # torch LSTM order i,f,g,o


def _build_program(T: int):
    och = min(OCH, T)
    NCH = T // och

    nc = bacc.Bacc(
        "TRN2", target_bir_lowering=False, debug=False, num_devices=NCORES
    )

    NBLK = -(-T // 4)  # 4 timesteps per column block (quads at partition 0/32/64/96)
    d_xq = [
        nc.dram_tensor(f"xq{g}", [128, NBLK * BG], DT, kind="ExternalInput")
        for g in range(NG)
    ]
    d_W = {
        k: nc.dram_tensor(f"W{k}", [128, 128], DT, kind="ExternalInput")
        for k in GATE_ORDER
    }
    d_X = {
        k: nc.dram_tensor(f"X{k}", [128, 128], DT, kind="ExternalInput")
        for k in GATE_ORDER
    }
    d_fcw = nc.dram_tensor("FCW", [128, 1], DT, kind="ExternalInput")
    d_fcb = nc.dram_tensor("FCB", [128, 1], F32, kind="ExternalInput")
    d_out = nc.dram_tensor("out", [128, T], F32, kind="ExternalOutput")

    with tile.TileContext(nc) as tc:
        with (
            tc.tile_pool(name="const", bufs=1) as cp,
            tc.tile_pool(name="state", bufs=1) as sp,
            tc.tile_pool(name="work", bufs=3) as wp,
            tc.tile_pool(name="ps_g", bufs=2, space=bass.MemorySpace.PSUM) as pg,
            tc.tile_pool(name="ps_o", bufs=2, space=bass.MemorySpace.PSUM) as po,
        ):
            xqs = [cp.tile([128, NBLK * BG], DT, tag=f"xq{g}", name=f"xq{g}_sb") for g in range(NG)]
            Wsb = {k: cp.tile([128, 128], DT, tag=f"W{k}", name=f"W{k}_sb") for k in GATE_ORDER}
            Xsb = {k: cp.tile([128, 128], DT, tag=f"X{k}", name=f"X{k}_sb") for k in GATE_ORDER}
            fcw = cp.tile([128, 1], DT, tag="fcw")
            fcb = cp.tile([128, 1], F32, tag="fcb")
            outsb = cp.tile([128, T], F32, tag="outsb")

            for g in range(NG):
                nc.sync.dma_start(xqs[g][:], d_xq[g].ap())
            for k in GATE_ORDER:
                nc.sync.dma_start(Wsb[k][:], d_W[k].ap())
                nc.sync.dma_start(Xsb[k][:], d_X[k].ap())
            nc.sync.dma_start(fcw[:], d_fcw.ap())
            nc.sync.dma_start(fcb[:], d_fcb.ap())

            Hs = [sp.tile([128, BG], DT, tag=f"H{g}", name=f"H{g}_sb") for g in range(NG)]
            Cc = [sp.tile([128, BG], F32, tag=f"C{g}", name=f"C{g}_sb") for g in range(NG)]
            for g in range(NG):
                nc.gpsimd.memset(Hs[g][:], 0.0)
                nc.gpsimd.memset(Cc[g][:], 0.0)

            for c in range(NCH):
                pout = po.tile([128, och], F32, tag="pout", name=f"pout_{c}")
                for tt in range(och):
                    t = c * och + tt
                    blk, m = divmod(t, 4)
                    base = 32 * m
                    for g in range(NG):
                        ps = pg.tile([128, 4 * BG], F32, tag=f"ps{g}", name=f"ps{g}_{t}")
                        xr = xqs[g][base : base + 4, blk * BG : (blk + 1) * BG]
                        for j, k in enumerate(GATE_ORDER):
                            # input + bias contribution (no recurrence dep)
                            nc.tensor.matmul(
                                ps[:, j * BG : (j + 1) * BG],
                                Xsb[k][base : base + 4, :],
                                xr,
                                start=True,
                                stop=False,
                                tile_position=(base, 0),
                            )
                            # recurrent contribution
                            nc.tensor.matmul(
                                ps[:, j * BG : (j + 1) * BG],
                                Wsb[k][:],
                                Hs[g][:],
                                start=False,
                                stop=True,
                            )
                        S = wp.tile([128, 4 * BG], DT, tag=f"S{g}", name=f"S{g}_{t}")
                        nc.scalar.activation(S[:, 0 : 3 * BG], ps[:, 0 : 3 * BG], AF.Sigmoid)
                        nc.scalar.activation(
                            S[:, 3 * BG : 4 * BG], ps[:, 3 * BG : 4 * BG], AF.Tanh
                        )
                        P1 = wp.tile([128, BG], F32, tag=f"P1{g}", name=f"P1{g}_{t}")
                        P2 = wp.tile([128, BG], F32, tag=f"P2{g}", name=f"P2{g}_{t}")
                        nc.vector.tensor_mul(P1[:], S[:, 0:BG], S[:, 3 * BG : 4 * BG])
                        nc.vector.tensor_mul(P2[:], S[:, BG : 2 * BG], Cc[g][:])
                        nc.vector.tensor_add(Cc[g][:], P1[:], P2[:])
                        TC = wp.tile([128, BG], DT, tag=f"TC{g}", name=f"TC{g}_{t}")
                        nc.scalar.activation(TC[:], Cc[g][:], AF.Tanh)
                        nc.vector.tensor_mul(Hs[g][:], S[:, 2 * BG : 3 * BG], TC[:])
                        nc.tensor.matmul(
                            pout[g * BG : (g + 1) * BG, tt : tt + 1],
                            Hs[g][:],
                            fcw[:],
                            start=True,
                            stop=True,
                            tile_position=(0, g * BG),
                        )
                # fold in fc bias while draining psum -> SBUF [b, t] layout
                nc.vector.tensor_scalar_add(
                    outsb[:, c * och : (c + 1) * och], pout[:], fcb[:]
                )

            nc.sync.dma_start(d_out.ap(), outsb[:])

    nc.compile()
    return nc


_PROGRAM_CACHE: dict[int, object] = {}


def _get_program(T: int):
    if T not in _PROGRAM_CACHE:
        _PROGRAM_CACHE[T] = _build_program(T)
    return _PROGRAM_CACHE[T]


def _build_xq(xg: np.ndarray) -> np.ndarray:
    """xg: [BG, T] f32 -> [128, (T/4)*BG] bf16. Step t's quad
    [x_t; ones; x_rev_t; ones] sits at partition 32*(t%4), col block t//4."""
    BGl, T = xg.shape
    xgr = xg[:, ::-1]
    A = np.ones((T, 4, BGl), np.float32)
    A[:, 0, :] = xg.T
    A[:, 2, :] = xgr.T
    Tp = -(-T // 4) * 4                          # pad T up to a multiple of 4
    Ap = np.zeros((Tp, 4, BGl), np.float32)
    Ap[:T] = A
    A2 = Ap.reshape(Tp // 4, 4, 4, BGl)          # [blk, t%4, row, n]
    Z = np.zeros((4, 32, Tp // 4, BGl), np.float32)
    Z[:, 0:4] = A2.transpose(1, 2, 0, 3)         # [t%4, row, blk, n]
    return np.ascontiguousarray(Z.reshape(128, (Tp // 4) * BGl)).astype(BF16)


def _prep_weights(Wih_f, Whh_f, bih_f, bhh_f, Wih_b, Whh_b, bih_b, bhh_b, fc_w, fc_b):
    m = {}
    for k in GATE_ORDER:
        g0 = GATE_OFFSET[k]
        W = np.zeros((128, 128), np.float32)
        W[:64, :64] = Whh_f[g0 : g0 + 64, :].T
        W[64:, 64:] = Whh_b[g0 : g0 + 64, :].T
        m[f"W{k}"] = W.astype(BF16)
        X = np.zeros((128, 128), np.float32)
        for mm in range(4):
            X[32 * mm + 0, :64] = Wih_f[g0 : g0 + 64, 0]
            X[32 * mm + 1, :64] = bih_f[g0 : g0 + 64] + bhh_f[g0 : g0 + 64]
            X[32 * mm + 2, 64:] = Wih_b[g0 : g0 + 64, 0]
            X[32 * mm + 3, 64:] = bih_b[g0 : g0 + 64] + bhh_b[g0 : g0 + 64]
        m[f"X{k}"] = X.astype(BF16)
    m["FCW"] = fc_w.reshape(128, 1).astype(BF16)
    m["FCB"] = np.full((128, 1), float(np.asarray(fc_b).reshape(-1)[0]), np.float32)
    return m


def run(inputs: dict, trace: bool = False):
    x = np.asarray(inputs["x"], np.float32)
    B, T, _ = x.shape
    assert B == NCORES * BLOCAL and (T % OCH == 0 or OCH % T == 0), (B, T)

    common = _prep_weights(
        np.asarray(inputs["Wih_f"], np.float32),
        np.asarray(inputs["Whh_f"], np.float32),
        np.asarray(inputs["bih_f"], np.float32),
        np.asarray(inputs["bhh_f"], np.float32),
        np.asarray(inputs["Wih_b"], np.float32),
        np.asarray(inputs["Whh_b"], np.float32),
        np.asarray(inputs["bih_b"], np.float32),
        np.asarray(inputs["bhh_b"], np.float32),
        np.asarray(inputs["fc_w"], np.float32),
        np.asarray(inputs["fc_b"], np.float32),
    )

    in_maps = []
    for cid in range(NCORES):
        m = dict(common)
        xc = x[cid * BLOCAL : (cid + 1) * BLOCAL, :, 0]
        for g in range(NG):
            m[f"xq{g}"] = _build_xq(xc[g * BG : (g + 1) * BG])
        in_maps.append(m)

    nc = _get_program(T)
    res = run_bass_kernel_spmd(
        nc, in_maps, core_ids=list(range(NCORES)), trace=trace
    )
    out = np.concatenate(
        [res.results[i]["out"] for i in range(NCORES)], axis=0
    )  # [B, T]
    return out[..., None].astype(np.float32), res


def kernel(**inputs) -> np.ndarray:
    out, _ = run(inputs, trace=False)
    return out

